# revision 11
# baseline (speedup 1.0000x reference)
"""Trainium2 Bass kernel: LSTM + 2-layer GCN + mean-pool + MLP classifier.

Entire network runs on 8 NeuronCores in ONE fused SPMD program:
  - LSTM: replicated 64-batch recurrence in transposed layout (gates on
    partitions, batch on free dim); Wih applied inline per step; bias folded
    in with one wide DVE add; h captured at t = seq_len-1 (capture steps
    baked into the program at build time).
  - GCN: dst-rows sharded 8 ways (6272 rows/core).  Layer 1: every core
    computes the full (x @ W1.T) * dinv table (replicated dense matmul,
    bf16).  Edge aggregation: per 128-dst-row block, one dma_gather per
    src-half (int16 indices; table split at 32768) pulls all edge rows,
    then segment-matrix (is_equal vs iota) matmuls accumulate in PSUM.
    Layer 2: shard matmul of relu-ed h1 (PE-transposed), AllGather of the
    bf16 table, same aggregation, feeding a mean-pool matmul (graph counts
    folded in on host).
  - Host does only index preprocessing and the tiny 64-row classifier head.

LSTM steps are interleaved with GNN work units at emission time so the
serial recurrence latency hides under the DMA/matmul-heavy GNN phases.
"""

import numpy as np

B, T, DS, H = 64, 512, 128, 256
G4 = 4 * H
N, E = 50000, 1600000
D = 256
N_CORES = 8
P = 128
R = 6272
NP = N_CORES * R  # 50176
NBLK = R // P  # 49
MCHUNK = 896  # xT columns per SBUF chunk
NCHUNK = NP // MCHUNK  # 56
SQG = 64  # LSTM steps per seqs chunk
HALF = 32768  # table split for int16 gather indices

_CACHE = {}


# ---------------------------------------------------------------- host prep
def _graph_arrays(edge_index):
    src = np.concatenate([edge_index[0], np.arange(N, dtype=np.int64)])
    dst = np.concatenate([edge_index[1], np.arange(N, dtype=np.int64)])
    deg = np.bincount(dst, minlength=NP).astype(np.float32)
    dinv = np.zeros(NP, np.float32)
    nz = deg > 0
    dinv[nz] = 1.0 / np.sqrt(deg[nz])

    gblk = (dst // P).astype(np.int64)
    half = (src >= HALF).astype(np.int64)
    key = gblk * 2 + half
    order = np.argsort(key, kind="stable")
    src_s = src[order].astype(np.int32)
    dst_s = dst[order].astype(np.int32)
    key_s = key[order]

    counts = np.bincount(key_s, minlength=N_CORES * NBLK * 2)
    cnt = counts.reshape(N_CORES, NBLK * 2)
    # tiles per (local block, half), uniform across cores
    TKH = (cnt.max(axis=0).reshape(NBLK, 2) + P - 1) // P  # [49, 2]
    TKH = np.maximum(TKH, 1)
    NT = int(TKH.sum())
    toff = np.concatenate([[0], np.cumsum(TKH.reshape(-1))[:-1]])  # [98]

    group_start = np.concatenate([[0], np.cumsum(counts)[:-1]])
    within = np.arange(len(src_s)) - group_start[key_s]
    lgrp = key_s % (NBLK * 2)  # local (block, half) group id
    slot = (toff[lgrp] * P + within).astype(np.int64)

    esrc = np.zeros((N_CORES, NT * P), np.int32)
    edst = np.full((N_CORES, NT * P), 300.0, np.float32)
    core = key_s // (NBLK * 2)
    flat = core * (NT * P) + slot
    esrc.reshape(-1)[flat] = src_s - (src_s >= HALF).astype(np.int32) * HALF
    edst.reshape(-1)[flat] = (dst_s % P).astype(np.float32)

    # eidx: int16 wrap for dma_gather: per group, idx i -> [i%16, off*8 + i//16]
    eidx = np.zeros((N_CORES, 16, NT * 8), np.int16)
    es3 = esrc.reshape(N_CORES, NT, P)
    TKHf = TKH.reshape(-1)
    for g in range(NBLK * 2):
        o = int(toff[g])
        tkh = int(TKHf[g])
        fl = es3[:, o : o + tkh, :].reshape(N_CORES, tkh * P)  # i = t*128+p
        eidx[:, :, o * 8 : (o + tkh) * 8] = fl.reshape(
            N_CORES, tkh * 8, 16).transpose(0, 2, 1)
    eidx = np.ascontiguousarray(np.tile(eidx, (1, 8, 1)))  # [C, 128, NT*8]

    edst = np.ascontiguousarray(edst.reshape(N_CORES, NT, P).transpose(0, 2, 1))
    dinvd = np.ascontiguousarray(dinv.reshape(N_CORES, NBLK, P).transpose(0, 2, 1))
    return dinv, eidx, edst, dinvd, TKH, NT


# ---------------------------------------------------------------- program
def _build_nc(TKH, NT, seq_lens):
    import concourse.tile as tile
    from concourse import bacc, library_config, mybir

    fp32, bf16 = mybir.dt.float32, mybir.dt.bfloat16
    i16 = mybir.dt.int16
    SIG = mybir.ActivationFunctionType.Sigmoid
    TANH = mybir.ActivationFunctionType.Tanh
    COPY = mybir.ActivationFunctionType.Copy

    cap = [[] for _ in range(T)]
    for b, L in enumerate(seq_lens):
        cap[int(L) - 1].append(b)

    TKmax = int(TKH.max())
    toff = np.concatenate([[0], np.cumsum(TKH.reshape(-1))[:-1]]).astype(int)

    nc = bacc.Bacc("TRN2", target_bir_lowering=False, debug=False,
                   enable_asserts=False, num_devices=N_CORES)
    # ---- I/O ----
    seqsT_d = nc.dram_tensor("seqsT", [P, T * B], bf16, kind="ExternalInput").ap()
    whhT_d = nc.dram_tensor("whhT", [P, 2 * G4], bf16, kind="ExternalInput").ap()
    wihT_d = nc.dram_tensor("wihT", [P, G4], bf16, kind="ExternalInput").ap()
    lbias_d = nc.dram_tensor("lbias", [P, 512], fp32, kind="ExternalInput").ap()
    xT_d = nc.dram_tensor("xT", [2 * P, NP], bf16, kind="ExternalInput").ap()
    w1t_d = nc.dram_tensor("w1t", [P, 2 * D], bf16, kind="ExternalInput").ap()
    w2t_d = nc.dram_tensor("w2t", [P, 2 * D], bf16, kind="ExternalInput").ap()
    dinv_d = nc.dram_tensor("dinv", [P, NP // P], fp32, kind="ExternalInput").ap()
    dinvd_d = nc.dram_tensor("dinvd", [P, NBLK], fp32, kind="ExternalInput").ap()
    eidx_d = nc.dram_tensor("eidx", [P, NT * 8], i16, kind="ExternalInput").ap()
    edst_d = nc.dram_tensor("edst", [P, NT], bf16, kind="ExternalInput").ap()
    iota_d = nc.dram_tensor("iotaf", [P, P], bf16, kind="ExternalInput").ap()
    ident_d = nc.dram_tensor("identf", [P, P], bf16, kind="ExternalInput").ap()
    b1_d = nc.dram_tensor("b1r", [P, D], fp32, kind="ExternalInput").ap()
    b2_d = nc.dram_tensor("b2r", [P, D], fp32, kind="ExternalInput").ap()
    pmat_d = nc.dram_tensor("pmat", [P, NBLK * B], fp32, kind="ExternalInput").ap()
    hcap_d = nc.dram_tensor("hcap", [P, 2 * B], fp32, kind="ExternalOutput").ap()
    pool_d = nc.dram_tensor("pool", [B, D], fp32, kind="ExternalOutput").ap()
    # ---- internal DRAM ----
    xw1_d = nc.dram_tensor("xw1i", [NP, D], bf16, kind="Internal").ap()
    xw2s_d = nc.dram_tensor("xw2s", [R, D], bf16, kind="Internal").ap()
    xw2f_d = nc.dram_tensor("xw2f", [NP, D], bf16, kind="Internal",
                            addr_space="Shared").ap()

    with tile.TileContext(nc) as tc:
        with (
            tc.tile_pool(name="const", bufs=1) as cpool,
            tc.tile_pool(name="seqs", bufs=2) as spool,
            tc.tile_pool(name="lwork", bufs=4) as lwpool,
            tc.tile_pool(name="achunk", bufs=2) as apool,
            tc.tile_pool(name="xwout", bufs=6) as xwpool,
            tc.tile_pool(name="gath", bufs=4) as gpool,
            tc.tile_pool(name="mmat", bufs=8) as mpool,
            tc.tile_pool(name="hwork", bufs=4) as hpool,
            tc.tile_pool(name="psL", bufs=2, space="PSUM") as psL,
            tc.tile_pool(name="psD", bufs=2, space="PSUM") as psD,
            tc.tile_pool(name="psA", bufs=2, space="PSUM") as psA,
            tc.tile_pool(name="psT", bufs=1, space="PSUM") as psT,
            tc.tile_pool(name="psP", bufs=1, space="PSUM") as psP,
        ):
            # ======== constants ========
            whhT_sb = cpool.tile([P, 2 * G4], bf16, name="whhT_sb")
            nc.sync.dma_start(whhT_sb[:], whhT_d[:])
            wihT_sb = cpool.tile([P, G4], bf16, name="wihT_sb")
            nc.sync.dma_start(wihT_sb[:], wihT_d[:])
            lbias_sb = cpool.tile([P, 512], fp32, name="lbias_sb")
            nc.sync.dma_start(lbias_sb[:], lbias_d[:])
            w1t_sb = cpool.tile([P, 2 * D], bf16, name="w1t_sb")
            nc.sync.dma_start(w1t_sb[:], w1t_d[:])
            w2t_sb = cpool.tile([P, 2 * D], bf16, name="w2t_sb")
            nc.sync.dma_start(w2t_sb[:], w2t_d[:])
            dinv_sb = cpool.tile([P, NP // P], fp32, name="dinv_sb")
            nc.sync.dma_start(dinv_sb[:], dinv_d[:])
            dinvd_sb = cpool.tile([P, NBLK], fp32, name="dinvd_sb")
            nc.sync.dma_start(dinvd_sb[:], dinvd_d[:])
            eidx_sb = cpool.tile([P, NT * 8], i16, name="eidx_sb")
            nc.sync.dma_start(eidx_sb[:], eidx_d[:])
            edst_sb = cpool.tile([P, NT], bf16, name="edst_sb")
            nc.sync.dma_start(edst_sb[:], edst_d[:])
            iota_f = cpool.tile([P, P], bf16, name="iota_f")
            nc.sync.dma_start(iota_f[:], iota_d[:])
            ident = cpool.tile([P, P], bf16, name="ident")
            nc.sync.dma_start(ident[:], ident_d[:])
            b1_sb = cpool.tile([P, D], fp32, name="b1_sb")
            nc.sync.dma_start(b1_sb[:], b1_d[:])
            b2_sb = cpool.tile([P, D], fp32, name="b2_sb")
            nc.sync.dma_start(b2_sb[:], b2_d[:])
            pmat_sb = cpool.tile([P, NBLK * B], fp32, name="pmat_sb")
            nc.sync.dma_start(pmat_sb[:], pmat_d[:])
            nc.gpsimd.load_library(library_config.mlp)

            h1_sb = cpool.tile([P, NBLK * D], bf16, name="h1_sb")
            hT = cpool.tile([P, 2 * 64], bf16, name="hT")
            cS = cpool.tile([P, 2 * 64], fp32, name="cS")
            hcap = cpool.tile([P, 2 * B], fp32, name="hcap")
            nc.vector.memset(hT[:], 0.0)
            nc.vector.memset(cS[:], 0.0)
            nc.vector.memset(hcap[:], 0.0)
            pool_ps = psP.tile([B, D], fp32, name="pool_ps")

            # ======== GNN work-unit generator ========
            def agg_block(k, tbl_d, bias_sb, h1_out):
                    ps = psA.tile([P, D], fp32, name="ps_agg", tag="psagg")
                    tkl = int(TKH[k, 0])
                    tkh = int(TKH[k, 1])
                    parts = []
                    for hh, tk in ((0, tkl), (1, tkh)):
                        o = int(toff[2 * k + hh])
                        src_view = tbl_d[:] if hh == 0 else tbl_d[HALF:NP, :]
                        # dma_gather is only reliable up to 1024 indices/call
                        for s0 in range(0, tk, 8):
                            sn = min(8, tk - s0)
                            gk = gpool.tile([P, 8, D], bf16, name="gk", tag="gk")
                            nc.gpsimd.dma_gather(
                                out_ap=gk[:, :sn, :], in_ap=src_view,
                                idxs_ap=eidx_sb[:, (o + s0) * 8 : (o + s0 + sn) * 8],
                                num_idxs=sn * P, num_idxs_reg=sn * P, elem_size=D)
                            parts.append((o + s0, sn, gk))
                            yield
                    ntot = tkl + tkh
                    done = 0
                    for (o, tk, gk) in parts:
                        for t in range(tk):
                            col = o + t
                            mt = mpool.tile([P, P], bf16, name="m_t", tag="mt")
                            nc.vector.tensor_tensor(
                                out=mt[:],
                                in0=edst_sb[:, col : col + 1].to_broadcast([P, P]),
                                in1=iota_f[:], op=mybir.AluOpType.is_equal)
                            nc.tensor.matmul(ps[:], lhsT=mt[:], rhs=gk[:, t, :],
                                             start=(done == 0),
                                             stop=(done == ntot - 1))
                            done += 1
                            if done % 8 == 0:
                                yield
                    if h1_out:
                        hdst = h1_sb[:, k * D : (k + 1) * D]
                        nc.vector.tensor_scalar(
                            out=hdst, in0=ps[:], scalar1=dinvd_sb[:, k : k + 1],
                            scalar2=None, op0=mybir.AluOpType.mult)
                        nc.vector.tensor_add(hdst, hdst, bias_sb[:])
                        nc.vector.tensor_scalar_max(hdst, hdst, 0.0)
                    else:
                        h2t = hpool.tile([P, D], fp32, name="h2t", tag="h2t")
                        nc.vector.tensor_scalar(
                            out=h2t[:], in0=ps[:], scalar1=dinvd_sb[:, k : k + 1],
                            scalar2=None, op0=mybir.AluOpType.mult)
                        nc.vector.tensor_add(h2t[:], h2t[:], bias_sb[:])
                        nc.vector.tensor_scalar_max(h2t[:], h2t[:], 0.0)
                        nc.tensor.matmul(pool_ps[:],
                                         lhsT=pmat_sb[:, k * B : (k + 1) * B],
                                         rhs=h2t[:], start=(k == 0),
                                         stop=(k == NBLK - 1))
                    yield

            # ======== dense xw1 phase: emitted up front, un-interleaved ====
            for ch in range(NCHUNK):
                c0 = ch * MCHUNK
                a0 = apool.tile([P, MCHUNK], bf16, name="a0", tag="a0")
                a1 = apool.tile([P, MCHUNK], bf16, name="a1", tag="a1")
                nc.sync.dma_start(a0[:], xT_d[0:P, c0 : c0 + MCHUNK])
                nc.sync.dma_start(a1[:], xT_d[P : 2 * P, c0 : c0 + MCHUNK])
                for m in range(MCHUNK // P):
                    js = slice(m * P, (m + 1) * P)
                    ps = psD.tile([P, D], fp32, name="ps_mm", tag="psmm")
                    nc.tensor.matmul(ps[:], lhsT=a0[:, js], rhs=w1t_sb[:, 0:D],
                                     start=True, stop=False)
                    nc.tensor.matmul(ps[:], lhsT=a1[:, js],
                                     rhs=w1t_sb[:, D : 2 * D],
                                     start=False, stop=True)
                    ot = xwpool.tile([P, D], bf16, name="xw_t", tag="xwt")
                    gm = c0 // P + m
                    nc.scalar.activation(ot[:], ps[:], COPY, bias=0.0,
                                         scale=dinv_sb[:, gm : gm + 1])
                    nc.sync.dma_start(xw1_d[gm * P : (gm + 1) * P, :], ot[:])

            def gnn_units():
                # --- agg layer 1, with block-k xw2 transpose/matmul woven in
                #     so the AllGather can fire as soon as agg1 finishes ---
                for k in range(NBLK):
                    yield from agg_block(k, xw1_d, b1_sb, h1_out=True)
                    ps2 = psD.tile([P, D], fp32, name="ps_x2", tag="psmm")
                    for half in range(2):
                        tp = psT.tile([P, P], bf16, name="tp", tag="tp")
                        nc.tensor.transpose(
                            tp[:],
                            h1_sb[:, k * D + half * P : k * D + (half + 1) * P],
                            ident[:])
                        h1t = hpool.tile([P, P], bf16, name="h1t", tag="h1t")
                        nc.vector.tensor_copy(h1t[:], tp[:])
                        nc.tensor.matmul(ps2[:], lhsT=h1t[:],
                                         rhs=w2t_sb[:, half * D : (half + 1) * D],
                                         start=(half == 0), stop=(half == 1))
                    ot2 = xwpool.tile([P, D], bf16, name="xw2_t", tag="xwt")
                    nc.scalar.activation(ot2[:], ps2[:], COPY, bias=0.0,
                                         scale=dinvd_sb[:, k : k + 1])
                    nc.sync.dma_start(xw2s_d[k * P : (k + 1) * P, :], ot2[:])
                    yield

                # --- allgather xw2 ---
                nc.gpsimd.collective_compute(
                    "AllGather", mybir.AluOpType.bypass,
                    replica_groups=[list(range(N_CORES))],
                    ins=[xw2s_d[:]], outs=[xw2f_d[:]])
                yield

                # --- aggregation layer 2 + pool ---
                for k in range(NBLK):
                    yield from agg_block(k, xw2f_d, b2_sb, h1_out=False)

                pool_sb = cpool.tile([B, D], fp32, name="pool_sb")
                nc.vector.tensor_copy(pool_sb[:], pool_ps[:])
                nc.sync.dma_start(pool_d[:], pool_sb[:])
                yield

            # ======== interleaved emission: LSTM steps + GNN units ========
            gnn = gnn_units()
            n_groups = int(sum((int(t_) + 7) // 8 for t_ in TKH.reshape(-1)))
            n_units = 2 * (n_groups + (NT + 7) // 8 + NBLK) + NBLK + 2
            rate = n_units / float(T)
            credit = 0.0

            gnn_done = False
            for t in range(T):
                g, lt = divmod(t, SQG)
                if lt == 0:
                    sq = spool.tile([P, SQG * B], bf16, name="sq", tag="sq")
                    nc.sync.dma_start(sq[:],
                                      seqsT_d[:, g * SQG * B : (g + 1) * SQG * B])
                ps = psL.tile([P, 512], fp32, name="ps_g", tag="psg")
                for c in range(8):
                    cs = slice(c * 64, (c + 1) * 64)
                    nc.tensor.matmul(ps[:, cs],
                                     lhsT=whhT_sb[:, c * P : (c + 1) * P],
                                     rhs=hT[:, 0:64], start=True, stop=False)
                    nc.tensor.matmul(ps[:, cs],
                                     lhsT=whhT_sb[:, G4 + c * P : G4 + (c + 1) * P],
                                     rhs=hT[:, 64:128], start=False, stop=False)
                    nc.tensor.matmul(ps[:, cs],
                                     lhsT=wihT_sb[:, c * P : (c + 1) * P],
                                     rhs=sq[:, lt * B : (lt + 1) * B],
                                     start=False, stop=True)
                gpre = lwpool.tile([P, 512], bf16, name="gpre", tag="gpre")
                nc.vector.tensor_add(gpre[:], ps[:], lbias_sb[:])
                gsb = lwpool.tile([P, 512], bf16, name="gsb", tag="gsb")
                nc.scalar.activation(gsb[:, 0:256], gpre[:, 0:256], SIG,
                                     bias=0.0, scale=1.0)
                nc.scalar.activation(gsb[:, 256:384], gpre[:, 256:384], TANH,
                                     bias=0.0, scale=1.0)
                nc.scalar.activation(gsb[:, 384:512], gpre[:, 384:512], SIG,
                                     bias=0.0, scale=1.0)
                t1 = lwpool.tile([P, 128], fp32, name="t1", tag="t1")
                nc.vector.tensor_mul(t1[:], gsb[:, 128:256], cS[:])
                t2 = lwpool.tile([P, 128], bf16, name="t2", tag="t2")
                nc.vector.tensor_mul(t2[:], gsb[:, 0:128], gsb[:, 256:384])
                nc.vector.tensor_add(cS[:], t1[:], t2[:])
                th = lwpool.tile([P, 128], bf16, name="th", tag="th")
                nc.scalar.activation(th[:], cS[:], TANH, bias=0.0, scale=1.0)
                nc.vector.tensor_mul(hT[:], gsb[:, 384:512], th[:])
                for b in cap[t]:
                    nc.vector.tensor_copy(hcap[:, b : b + 1], hT[:, b : b + 1])
                    nc.vector.tensor_copy(hcap[:, 64 + b : 65 + b],
                                          hT[:, 64 + b : 65 + b])
                if not gnn_done:
                    credit += rate
                    while credit >= 1.0:
                        credit -= 1.0
                        try:
                            next(gnn)
                        except StopIteration:
                            gnn_done = True
                            break
            while not gnn_done:
                try:
                    next(gnn)
                except StopIteration:
                    gnn_done = True
            nc.sync.dma_start(hcap_d[:], hcap[:])
    nc.compile()
    return nc


# ---------------------------------------------------------------- runner
def _prepare(seqs, seq_lens, x, edge_index, Wih, Whh, bih, bhh, W1, b1, W2, b2,
             batch):
    import ml_dtypes

    bf = lambda a: np.ascontiguousarray(a.astype(ml_dtypes.bfloat16))

    dinv, eidx, edst, dinvd, TKH, NT = _graph_arrays(edge_index)

    xp = np.zeros((NP, D), np.float32)
    xp[:N] = x
    xT = bf(xp.T)

    seqsT = bf(np.transpose(seqs, (2, 1, 0)).reshape(DS, T * B))
    whhT = bf(np.concatenate([Whh.T[:P, :], Whh.T[P:, :]], axis=1))
    wihT = bf(Wih.T)
    bias = bih + bhh
    lb = np.ascontiguousarray(
        (bias.reshape(8, P).T[:, :, None] * np.ones((1, 1, 64), np.float32))
        .reshape(P, 512))
    w1t = bf(np.concatenate([W1.T[:P, :], W1.T[P:, :]], axis=1))
    w2t = bf(np.concatenate([W2.T[:P, :], W2.T[P:, :]], axis=1))

    cnt = np.bincount(batch, minlength=B).astype(np.float32)
    inv_cnt = 1.0 / np.maximum(cnt, 1.0)
    bpad = np.full(NP, -1, np.int64)
    bpad[:N] = batch
    pm = np.zeros((NP, B), np.float32)
    valid = bpad >= 0
    pm[np.arange(NP)[valid], bpad[valid]] = inv_cnt[bpad[valid]]
    pmat = np.ascontiguousarray(
        pm.reshape(N_CORES, NBLK, P, B).transpose(0, 2, 1, 3).reshape(
            N_CORES, P, NBLK * B))

    iotaf = np.arange(P, dtype=np.float32)[None, :] * np.ones((P, 1), np.float32)
    identf = np.eye(P, dtype=np.float32)

    common = {
        "seqsT": seqsT, "whhT": whhT, "wihT": wihT, "lbias": lb,
        "xT": xT, "w1t": w1t, "w2t": w2t,
        "dinv": np.ascontiguousarray(dinv.reshape(NP // P, P).T),
        "iotaf": bf(iotaf), "identf": bf(identf),
        "b1r": np.ascontiguousarray(np.broadcast_to(b1, (P, D)).copy()),
        "b2r": np.ascontiguousarray(np.broadcast_to(b2, (P, D)).copy()),
    }
    in_maps = []
    for c in range(N_CORES):
        m = dict(common)
        m["dinvd"] = dinvd[c]
        m["eidx"] = eidx[c]
        m["edst"] = np.ascontiguousarray(edst[c].astype(ml_dtypes.bfloat16))
        m["pmat"] = pmat[c]
        in_maps.append(m)
    return in_maps, TKH, NT


def kernel(seqs, seq_lens, x, edge_index, batch, Wih, Whh, bih, bhh,
           W1, b1, W2, b2, Wc1, bc1, Wc2, bc2):
    seqs = np.asarray(seqs, np.float32)
    seq_lens = np.asarray(seq_lens).astype(np.int64)
    x = np.asarray(x, np.float32)
    edge_index = np.asarray(edge_index).astype(np.int64)
    batch = np.asarray(batch).astype(np.int64)
    (Wih, Whh, bih, bhh, W1, b1, W2, b2, Wc1, bc1, Wc2, bc2) = (
        np.asarray(a, np.float32)
        for a in (Wih, Whh, bih, bhh, W1, b1, W2, b2, Wc1, bc1, Wc2, bc2))

    in_maps, TKH, NT = _prepare(seqs, seq_lens, x, edge_index, Wih, Whh, bih,
                                bhh, W1, b1, W2, b2, batch)

    sig = (TKH.tobytes(), seq_lens.tobytes())
    if _CACHE.get("sig") != sig:
        _CACHE["nc"] = _build_nc(TKH, NT, seq_lens)
        _CACHE["sig"] = sig
    nc = _CACHE["nc"]

    from concourse.bass_utils import run_bass_kernel_spmd

    res = run_bass_kernel_spmd(nc, in_maps, list(range(N_CORES)))
    _CACHE["last"] = (nc, in_maps)
    return _assemble(res.results, Wc1, bc1, Wc2, bc2)


def _assemble(results, Wc1, bc1, Wc2, bc2):
    hc = np.asarray(results[0]["hcap"], np.float32)  # [128, 128]
    h_lstm = np.concatenate([hc[:, :B].T, hc[:, B:].T], axis=1)  # [64, 256]
    h_gnn = np.zeros((B, D), np.float32)
    for c in range(N_CORES):
        h_gnn += np.asarray(results[c]["pool"], np.float32)
    fused = np.concatenate([h_lstm, h_gnn], axis=1)
    z = np.maximum(fused @ Wc1.T + bc1, 0.0)
    return (z @ Wc2.T + bc2).astype(np.float32)


def profile_exec():
    """Re-run the last program with NTFF tracing; return BassKernelResults."""
    import profhook

    profhook.install()
    from concourse.bass_utils import run_bass_kernel_spmd

    nc, in_maps = _CACHE["last"]
    res = run_bass_kernel_spmd(nc, in_maps, list(range(N_CORES)), trace=True,
                               trace_cores=list(range(N_CORES)))
    return res


# revision 12
# speedup vs baseline: 1.2386x; 1.2386x over previous
"""Trainium2 Bass kernel: LSTM + 2-layer GCN + mean-pool + MLP classifier.

Entire network runs on 8 NeuronCores in ONE fused SPMD program:
  - LSTM: replicated 64-batch recurrence in transposed layout (gates on
    partitions, batch on free dim); Wih applied inline per step; bias folded
    in with one wide DVE add; h captured at t = seq_len-1 (capture steps
    baked into the program at build time).
  - GCN: dst-rows sharded 8 ways (6272 rows/core).  Layer 1: every core
    computes the full (x @ W1.T) * dinv table (replicated dense matmul,
    bf16).  Edge aggregation: per 128-dst-row block, one dma_gather per
    src-half (int16 indices; table split at 32768) pulls all edge rows,
    then segment-matrix (is_equal vs iota) matmuls accumulate in PSUM.
    Layer 2: shard matmul of relu-ed h1 (PE-transposed), AllGather of the
    bf16 table, same aggregation, feeding a mean-pool matmul (graph counts
    folded in on host).
  - Host does only index preprocessing and the tiny 64-row classifier head.

LSTM steps are interleaved with GNN work units at emission time so the
serial recurrence latency hides under the DMA/matmul-heavy GNN phases.
"""

import numpy as np

B, T, DS, H = 64, 512, 128, 256
G4 = 4 * H
N, E = 50000, 1600000
D = 256
N_CORES = 8
P = 128
R = 6272
NP = N_CORES * R  # 50176
NBLK = R // P  # 49
MCHUNK = 896  # xT columns per SBUF chunk
NCHUNK = NP // MCHUNK  # 56
SQG = 64  # LSTM steps per seqs chunk
HALF = 32768  # table split for int16 gather indices

_CACHE = {}


# ---------------------------------------------------------------- host prep
def _graph_arrays(edge_index):
    src = np.concatenate([edge_index[0], np.arange(N, dtype=np.int64)])
    dst = np.concatenate([edge_index[1], np.arange(N, dtype=np.int64)])
    deg = np.bincount(dst, minlength=NP).astype(np.float32)
    dinv = np.zeros(NP, np.float32)
    nz = deg > 0
    dinv[nz] = 1.0 / np.sqrt(deg[nz])

    gblk = (dst // P).astype(np.int64)
    half = (src >= HALF).astype(np.int64)
    key = gblk * 2 + half
    order = np.argsort(key, kind="stable")
    src_s = src[order].astype(np.int32)
    dst_s = dst[order].astype(np.int32)
    key_s = key[order]

    counts = np.bincount(key_s, minlength=N_CORES * NBLK * 2)
    cnt = counts.reshape(N_CORES, NBLK * 2)
    # tiles per (local block, half), uniform across cores
    TKH = (cnt.max(axis=0).reshape(NBLK, 2) + P - 1) // P  # [49, 2]
    TKH = np.maximum(TKH, 1)
    NT = int(TKH.sum())
    toff = np.concatenate([[0], np.cumsum(TKH.reshape(-1))[:-1]])  # [98]

    group_start = np.concatenate([[0], np.cumsum(counts)[:-1]])
    within = np.arange(len(src_s)) - group_start[key_s]
    lgrp = key_s % (NBLK * 2)  # local (block, half) group id
    slot = (toff[lgrp] * P + within).astype(np.int64)

    esrc = np.zeros((N_CORES, NT * P), np.int32)
    edst = np.full((N_CORES, NT * P), 300.0, np.float32)
    core = key_s // (NBLK * 2)
    flat = core * (NT * P) + slot
    esrc.reshape(-1)[flat] = src_s - (src_s >= HALF).astype(np.int32) * HALF
    edst.reshape(-1)[flat] = (dst_s % P).astype(np.float32)

    # eidx: int16 wrap for dma_gather: per group, idx i -> [i%16, off*8 + i//16]
    eidx = np.zeros((N_CORES, 16, NT * 8), np.int16)
    es3 = esrc.reshape(N_CORES, NT, P)
    TKHf = TKH.reshape(-1)
    for g in range(NBLK * 2):
        o = int(toff[g])
        tkh = int(TKHf[g])
        fl = es3[:, o : o + tkh, :].reshape(N_CORES, tkh * P)  # i = t*128+p
        eidx[:, :, o * 8 : (o + tkh) * 8] = fl.reshape(
            N_CORES, tkh * 8, 16).transpose(0, 2, 1)
    eidx = np.ascontiguousarray(np.tile(eidx, (1, 8, 1)))  # [C, 128, NT*8]

    edst = np.ascontiguousarray(edst.reshape(N_CORES, NT, P).transpose(0, 2, 1))
    dinvd = np.ascontiguousarray(dinv.reshape(N_CORES, NBLK, P).transpose(0, 2, 1))
    return dinv, eidx, edst, dinvd, TKH, NT


# ---------------------------------------------------------------- program
def _build_nc(TKH, NT, seq_lens):
    import concourse.tile as tile
    from concourse import bacc, library_config, mybir

    fp32, bf16 = mybir.dt.float32, mybir.dt.bfloat16
    i16 = mybir.dt.int16
    SIG = mybir.ActivationFunctionType.Sigmoid
    TANH = mybir.ActivationFunctionType.Tanh
    COPY = mybir.ActivationFunctionType.Copy

    cap = [[] for _ in range(T)]
    for b, L in enumerate(seq_lens):
        cap[int(L) - 1].append(b)

    TKmax = int(TKH.max())
    toff = np.concatenate([[0], np.cumsum(TKH.reshape(-1))[:-1]]).astype(int)

    nc = bacc.Bacc("TRN2", target_bir_lowering=False, debug=False,
                   enable_asserts=False, num_devices=N_CORES)
    # ---- I/O ----
    seqsT_d = nc.dram_tensor("seqsT", [P, T * B], bf16, kind="ExternalInput").ap()
    whhT_d = nc.dram_tensor("whhT", [P, 2 * G4], bf16, kind="ExternalInput").ap()
    wihT_d = nc.dram_tensor("wihT", [P, G4], bf16, kind="ExternalInput").ap()
    lbias_d = nc.dram_tensor("lbias", [P, 512], fp32, kind="ExternalInput").ap()
    xT_d = nc.dram_tensor("xT", [2 * P, NP], bf16, kind="ExternalInput").ap()
    w1t_d = nc.dram_tensor("w1t", [P, 2 * D], bf16, kind="ExternalInput").ap()
    w2t_d = nc.dram_tensor("w2t", [P, 2 * D], bf16, kind="ExternalInput").ap()
    dinv_d = nc.dram_tensor("dinv", [P, NP // P], fp32, kind="ExternalInput").ap()
    dinvd_d = nc.dram_tensor("dinvd", [P, NBLK], fp32, kind="ExternalInput").ap()
    eidx_d = nc.dram_tensor("eidx", [P, NT * 8], i16, kind="ExternalInput").ap()
    edst_d = nc.dram_tensor("edst", [P, NT], bf16, kind="ExternalInput").ap()
    iota_d = nc.dram_tensor("iotaf", [P, P], bf16, kind="ExternalInput").ap()
    ident_d = nc.dram_tensor("identf", [P, P], bf16, kind="ExternalInput").ap()
    b1_d = nc.dram_tensor("b1r", [P, D], fp32, kind="ExternalInput").ap()
    b2_d = nc.dram_tensor("b2r", [P, D], fp32, kind="ExternalInput").ap()
    pmat_d = nc.dram_tensor("pmat", [P, NBLK * B], fp32, kind="ExternalInput").ap()
    hcap_d = nc.dram_tensor("hcap", [P, 2 * B], fp32, kind="ExternalOutput").ap()
    pool_d = nc.dram_tensor("pool", [B, D], fp32, kind="ExternalOutput").ap()
    # ---- internal DRAM ----
    xw1_d = nc.dram_tensor("xw1i", [NP, D], bf16, kind="Internal").ap()
    xw2s_d = nc.dram_tensor("xw2s", [R, D], bf16, kind="Internal").ap()
    xw2f_d = nc.dram_tensor("xw2f", [NP, D], bf16, kind="Internal",
                            addr_space="Shared").ap()

    with tile.TileContext(nc) as tc:
        with (
            tc.tile_pool(name="const", bufs=1) as cpool,
            tc.tile_pool(name="seqs", bufs=2) as spool,
            tc.tile_pool(name="lwork", bufs=4) as lwpool,
            tc.tile_pool(name="achunk", bufs=2) as apool,
            tc.tile_pool(name="xwout", bufs=6) as xwpool,
            tc.tile_pool(name="gath", bufs=4) as gpool,
            tc.tile_pool(name="mmat", bufs=8) as mpool,
            tc.tile_pool(name="hwork", bufs=4) as hpool,
            tc.tile_pool(name="psL", bufs=2, space="PSUM") as psL,
            tc.tile_pool(name="psD", bufs=2, space="PSUM") as psD,
            tc.tile_pool(name="psA", bufs=2, space="PSUM") as psA,
            tc.tile_pool(name="psT", bufs=1, space="PSUM") as psT,
            tc.tile_pool(name="psP", bufs=1, space="PSUM") as psP,
        ):
            # ======== constants ========
            whhT_sb = cpool.tile([P, 2 * G4], bf16, name="whhT_sb")
            nc.sync.dma_start(whhT_sb[:], whhT_d[:])
            wihT_sb = cpool.tile([P, G4], bf16, name="wihT_sb")
            nc.sync.dma_start(wihT_sb[:], wihT_d[:])
            lbias_sb = cpool.tile([P, 512], fp32, name="lbias_sb")
            nc.sync.dma_start(lbias_sb[:], lbias_d[:])
            w1t_sb = cpool.tile([P, 2 * D], bf16, name="w1t_sb")
            nc.sync.dma_start(w1t_sb[:], w1t_d[:])
            w2t_sb = cpool.tile([P, 2 * D], bf16, name="w2t_sb")
            nc.sync.dma_start(w2t_sb[:], w2t_d[:])
            dinv_sb = cpool.tile([P, NP // P], fp32, name="dinv_sb")
            nc.sync.dma_start(dinv_sb[:], dinv_d[:])
            dinvd_sb = cpool.tile([P, NBLK], fp32, name="dinvd_sb")
            nc.sync.dma_start(dinvd_sb[:], dinvd_d[:])
            eidx_sb = cpool.tile([P, NT * 8], i16, name="eidx_sb")
            nc.sync.dma_start(eidx_sb[:], eidx_d[:])
            edst_sb = cpool.tile([P, NT], bf16, name="edst_sb")
            nc.sync.dma_start(edst_sb[:], edst_d[:])
            iota_f = cpool.tile([P, P], bf16, name="iota_f")
            nc.sync.dma_start(iota_f[:], iota_d[:])
            ident = cpool.tile([P, P], bf16, name="ident")
            nc.sync.dma_start(ident[:], ident_d[:])
            b1_sb = cpool.tile([P, D], fp32, name="b1_sb")
            nc.sync.dma_start(b1_sb[:], b1_d[:])
            b2_sb = cpool.tile([P, D], fp32, name="b2_sb")
            nc.sync.dma_start(b2_sb[:], b2_d[:])
            pmat_sb = cpool.tile([P, NBLK * B], fp32, name="pmat_sb")
            nc.sync.dma_start(pmat_sb[:], pmat_d[:])
            nc.gpsimd.load_library(library_config.mlp)

            h1_sb = cpool.tile([P, NBLK * D], bf16, name="h1_sb")
            hT = cpool.tile([P, 2 * 64], bf16, name="hT")
            cS = cpool.tile([P, 2 * 64], fp32, name="cS")
            hcap = cpool.tile([P, 2 * B], fp32, name="hcap")
            nc.vector.memset(hT[:], 0.0)
            nc.vector.memset(cS[:], 0.0)
            nc.vector.memset(hcap[:], 0.0)
            pool_ps = psP.tile([B, D], fp32, name="pool_ps")

            # ======== GNN work-unit generator ========
            def agg_layer(tbl_d, bias_sb, h1_out):
                for k in range(NBLK):
                    ps = psA.tile([P, D], fp32, name="ps_agg", tag="psagg")
                    tkl = int(TKH[k, 0])
                    tkh = int(TKH[k, 1])
                    parts = []
                    for hh, tk in ((0, tkl), (1, tkh)):
                        o = int(toff[2 * k + hh])
                        src_view = tbl_d[:] if hh == 0 else tbl_d[HALF:NP, :]
                        # dma_gather is only reliable up to 1024 indices/call
                        for s0 in range(0, tk, 8):
                            sn = min(8, tk - s0)
                            gk = gpool.tile([P, 8, D], bf16, name="gk", tag="gk")
                            nc.gpsimd.dma_gather(
                                out_ap=gk[:, :sn, :], in_ap=src_view,
                                idxs_ap=eidx_sb[:, (o + s0) * 8 : (o + s0 + sn) * 8],
                                num_idxs=sn * P, num_idxs_reg=sn * P, elem_size=D)
                            parts.append((o + s0, sn, gk))
                            yield
                    ntot = tkl + tkh
                    done = 0
                    for (o, tk, gk) in parts:
                        for t in range(tk):
                            col = o + t
                            mt = mpool.tile([P, P], bf16, name="m_t", tag="mt")
                            nc.vector.tensor_tensor(
                                out=mt[:],
                                in0=edst_sb[:, col : col + 1].to_broadcast([P, P]),
                                in1=iota_f[:], op=mybir.AluOpType.is_equal)
                            nc.tensor.matmul(ps[:], lhsT=mt[:], rhs=gk[:, t, :],
                                             start=(done == 0),
                                             stop=(done == ntot - 1))
                            done += 1
                            if done % 8 == 0:
                                yield
                    if h1_out:
                        hdst = h1_sb[:, k * D : (k + 1) * D]
                        nc.vector.tensor_scalar(
                            out=hdst, in0=ps[:], scalar1=dinvd_sb[:, k : k + 1],
                            scalar2=None, op0=mybir.AluOpType.mult)
                        nc.vector.tensor_add(hdst, hdst, bias_sb[:])
                        nc.vector.tensor_scalar_max(hdst, hdst, 0.0)
                    else:
                        h2t = hpool.tile([P, D], fp32, name="h2t", tag="h2t")
                        nc.vector.tensor_scalar(
                            out=h2t[:], in0=ps[:], scalar1=dinvd_sb[:, k : k + 1],
                            scalar2=None, op0=mybir.AluOpType.mult)
                        nc.vector.tensor_add(h2t[:], h2t[:], bias_sb[:])
                        nc.vector.tensor_scalar_max(h2t[:], h2t[:], 0.0)
                        nc.tensor.matmul(pool_ps[:],
                                         lhsT=pmat_sb[:, k * B : (k + 1) * B],
                                         rhs=h2t[:], start=(k == 0),
                                         stop=(k == NBLK - 1))
                    yield

            def gnn_units():
                # --- dense xw1 over all NP rows ---
                for ch in range(NCHUNK):
                    c0 = ch * MCHUNK
                    a0 = apool.tile([P, MCHUNK], bf16, name="a0", tag="a0")
                    a1 = apool.tile([P, MCHUNK], bf16, name="a1", tag="a1")
                    nc.sync.dma_start(a0[:], xT_d[0:P, c0 : c0 + MCHUNK])
                    nc.sync.dma_start(a1[:], xT_d[P : 2 * P, c0 : c0 + MCHUNK])
                    for m in range(MCHUNK // P):
                        js = slice(m * P, (m + 1) * P)
                        ps = psD.tile([P, D], fp32, name="ps_mm", tag="psmm")
                        nc.tensor.matmul(ps[:], lhsT=a0[:, js], rhs=w1t_sb[:, 0:D],
                                         start=True, stop=False)
                        nc.tensor.matmul(ps[:], lhsT=a1[:, js],
                                         rhs=w1t_sb[:, D : 2 * D],
                                         start=False, stop=True)
                        ot = xwpool.tile([P, D], bf16, name="xw_t", tag="xwt")
                        gm = c0 // P + m
                        nc.scalar.activation(ot[:], ps[:], COPY, bias=0.0,
                                             scale=dinv_sb[:, gm : gm + 1])
                        nc.sync.dma_start(xw1_d[gm * P : (gm + 1) * P, :], ot[:])
                        yield

                # --- aggregation layer 1 -> h1 resident ---
                yield from agg_layer(xw1_d, b1_sb, h1_out=True)

                # --- transpose h1 + xw2 shard matmul ---
                for k in range(NBLK):
                    ps2 = psD.tile([P, D], fp32, name="ps_x2", tag="psmm")
                    for half in range(2):
                        tp = psT.tile([P, P], bf16, name="tp", tag="tp")
                        nc.tensor.transpose(
                            tp[:],
                            h1_sb[:, k * D + half * P : k * D + (half + 1) * P],
                            ident[:])
                        h1t = hpool.tile([P, P], bf16, name="h1t", tag="h1t")
                        nc.vector.tensor_copy(h1t[:], tp[:])
                        nc.tensor.matmul(ps2[:], lhsT=h1t[:],
                                         rhs=w2t_sb[:, half * D : (half + 1) * D],
                                         start=(half == 0), stop=(half == 1))
                    ot2 = xwpool.tile([P, D], bf16, name="xw2_t", tag="xwt")
                    nc.scalar.activation(ot2[:], ps2[:], COPY, bias=0.0,
                                         scale=dinvd_sb[:, k : k + 1])
                    nc.sync.dma_start(xw2s_d[k * P : (k + 1) * P, :], ot2[:])
                    yield

                # --- allgather xw2 ---
                nc.gpsimd.collective_compute(
                    "AllGather", mybir.AluOpType.bypass,
                    replica_groups=[list(range(N_CORES))],
                    ins=[xw2s_d[:]], outs=[xw2f_d[:]])
                yield

                # --- aggregation layer 2 + pool ---
                yield from agg_layer(xw2f_d, b2_sb, h1_out=False)

                pool_sb = cpool.tile([B, D], fp32, name="pool_sb")
                nc.vector.tensor_copy(pool_sb[:], pool_ps[:])
                nc.sync.dma_start(pool_d[:], pool_sb[:])
                yield

            # ======== interleaved emission: LSTM steps + GNN units ========
            gnn = gnn_units()
            n_units = (NCHUNK * (MCHUNK // P) + NBLK
                       + 2 * (2 * NBLK + (NT + 7) // 8 + NBLK) + 3)
            per_step = max(1, (n_units + T - 1) // T)

            gnn_done = False
            for t in range(T):
                g, lt = divmod(t, SQG)
                if lt == 0:
                    sq = spool.tile([P, SQG * B], bf16, name="sq", tag="sq")
                    nc.sync.dma_start(sq[:],
                                      seqsT_d[:, g * SQG * B : (g + 1) * SQG * B])
                ps = psL.tile([P, 512], fp32, name="ps_g", tag="psg")
                for c in range(8):
                    cs = slice(c * 64, (c + 1) * 64)
                    nc.tensor.matmul(ps[:, cs],
                                     lhsT=whhT_sb[:, c * P : (c + 1) * P],
                                     rhs=hT[:, 0:64], start=True, stop=False)
                    nc.tensor.matmul(ps[:, cs],
                                     lhsT=whhT_sb[:, G4 + c * P : G4 + (c + 1) * P],
                                     rhs=hT[:, 64:128], start=False, stop=False)
                    nc.tensor.matmul(ps[:, cs],
                                     lhsT=wihT_sb[:, c * P : (c + 1) * P],
                                     rhs=sq[:, lt * B : (lt + 1) * B],
                                     start=False, stop=True)
                gpre = lwpool.tile([P, 512], bf16, name="gpre", tag="gpre")
                nc.vector.tensor_add(gpre[:], ps[:], lbias_sb[:])
                gsb = lwpool.tile([P, 512], bf16, name="gsb", tag="gsb")
                nc.scalar.activation(gsb[:, 0:256], gpre[:, 0:256], SIG,
                                     bias=0.0, scale=1.0)
                nc.scalar.activation(gsb[:, 256:384], gpre[:, 256:384], TANH,
                                     bias=0.0, scale=1.0)
                nc.scalar.activation(gsb[:, 384:512], gpre[:, 384:512], SIG,
                                     bias=0.0, scale=1.0)
                t1 = lwpool.tile([P, 128], fp32, name="t1", tag="t1")
                nc.vector.tensor_mul(t1[:], gsb[:, 128:256], cS[:])
                t2 = lwpool.tile([P, 128], bf16, name="t2", tag="t2")
                nc.vector.tensor_mul(t2[:], gsb[:, 0:128], gsb[:, 256:384])
                nc.vector.tensor_add(cS[:], t1[:], t2[:])
                th = lwpool.tile([P, 128], bf16, name="th", tag="th")
                nc.scalar.activation(th[:], cS[:], TANH, bias=0.0, scale=1.0)
                nc.vector.tensor_mul(hT[:], gsb[:, 384:512], th[:])
                for b in cap[t]:
                    nc.vector.tensor_copy(hcap[:, b : b + 1], hT[:, b : b + 1])
                    nc.vector.tensor_copy(hcap[:, 64 + b : 65 + b],
                                          hT[:, 64 + b : 65 + b])
                if not gnn_done:
                    for _ in range(per_step):
                        try:
                            next(gnn)
                        except StopIteration:
                            gnn_done = True
                            break
            while not gnn_done:
                try:
                    next(gnn)
                except StopIteration:
                    gnn_done = True
            nc.sync.dma_start(hcap_d[:], hcap[:])
    nc.compile()
    return nc


# ---------------------------------------------------------------- runner
def _prepare(seqs, seq_lens, x, edge_index, Wih, Whh, bih, bhh, W1, b1, W2, b2,
             batch):
    import ml_dtypes

    bf = lambda a: np.ascontiguousarray(a.astype(ml_dtypes.bfloat16))

    dinv, eidx, edst, dinvd, TKH, NT = _graph_arrays(edge_index)

    xp = np.zeros((NP, D), np.float32)
    xp[:N] = x
    xT = bf(xp.T)

    seqsT = bf(np.transpose(seqs, (2, 1, 0)).reshape(DS, T * B))
    whhT = bf(np.concatenate([Whh.T[:P, :], Whh.T[P:, :]], axis=1))
    wihT = bf(Wih.T)
    bias = bih + bhh
    lb = np.ascontiguousarray(
        (bias.reshape(8, P).T[:, :, None] * np.ones((1, 1, 64), np.float32))
        .reshape(P, 512))
    w1t = bf(np.concatenate([W1.T[:P, :], W1.T[P:, :]], axis=1))
    w2t = bf(np.concatenate([W2.T[:P, :], W2.T[P:, :]], axis=1))

    cnt = np.bincount(batch, minlength=B).astype(np.float32)
    inv_cnt = 1.0 / np.maximum(cnt, 1.0)
    bpad = np.full(NP, -1, np.int64)
    bpad[:N] = batch
    pm = np.zeros((NP, B), np.float32)
    valid = bpad >= 0
    pm[np.arange(NP)[valid], bpad[valid]] = inv_cnt[bpad[valid]]
    pmat = np.ascontiguousarray(
        pm.reshape(N_CORES, NBLK, P, B).transpose(0, 2, 1, 3).reshape(
            N_CORES, P, NBLK * B))

    iotaf = np.arange(P, dtype=np.float32)[None, :] * np.ones((P, 1), np.float32)
    identf = np.eye(P, dtype=np.float32)

    common = {
        "seqsT": seqsT, "whhT": whhT, "wihT": wihT, "lbias": lb,
        "xT": xT, "w1t": w1t, "w2t": w2t,
        "dinv": np.ascontiguousarray(dinv.reshape(NP // P, P).T),
        "iotaf": bf(iotaf), "identf": bf(identf),
        "b1r": np.ascontiguousarray(np.broadcast_to(b1, (P, D)).copy()),
        "b2r": np.ascontiguousarray(np.broadcast_to(b2, (P, D)).copy()),
    }
    in_maps = []
    for c in range(N_CORES):
        m = dict(common)
        m["dinvd"] = dinvd[c]
        m["eidx"] = eidx[c]
        m["edst"] = np.ascontiguousarray(edst[c].astype(ml_dtypes.bfloat16))
        m["pmat"] = pmat[c]
        in_maps.append(m)
    return in_maps, TKH, NT


def kernel(seqs, seq_lens, x, edge_index, batch, Wih, Whh, bih, bhh,
           W1, b1, W2, b2, Wc1, bc1, Wc2, bc2):
    seqs = np.asarray(seqs, np.float32)
    seq_lens = np.asarray(seq_lens).astype(np.int64)
    x = np.asarray(x, np.float32)
    edge_index = np.asarray(edge_index).astype(np.int64)
    batch = np.asarray(batch).astype(np.int64)
    (Wih, Whh, bih, bhh, W1, b1, W2, b2, Wc1, bc1, Wc2, bc2) = (
        np.asarray(a, np.float32)
        for a in (Wih, Whh, bih, bhh, W1, b1, W2, b2, Wc1, bc1, Wc2, bc2))

    in_maps, TKH, NT = _prepare(seqs, seq_lens, x, edge_index, Wih, Whh, bih,
                                bhh, W1, b1, W2, b2, batch)

    sig = (TKH.tobytes(), seq_lens.tobytes())
    if _CACHE.get("sig") != sig:
        _CACHE["nc"] = _build_nc(TKH, NT, seq_lens)
        _CACHE["sig"] = sig
    nc = _CACHE["nc"]

    from concourse.bass_utils import run_bass_kernel_spmd

    res = run_bass_kernel_spmd(nc, in_maps, list(range(N_CORES)))
    _CACHE["last"] = (nc, in_maps)
    return _assemble(res.results, Wc1, bc1, Wc2, bc2)


def _assemble(results, Wc1, bc1, Wc2, bc2):
    hc = np.asarray(results[0]["hcap"], np.float32)  # [128, 128]
    h_lstm = np.concatenate([hc[:, :B].T, hc[:, B:].T], axis=1)  # [64, 256]
    h_gnn = np.zeros((B, D), np.float32)
    for c in range(N_CORES):
        h_gnn += np.asarray(results[c]["pool"], np.float32)
    fused = np.concatenate([h_lstm, h_gnn], axis=1)
    z = np.maximum(fused @ Wc1.T + bc1, 0.0)
    return (z @ Wc2.T + bc2).astype(np.float32)


def profile_exec():
    """Re-run the last program with NTFF tracing; return BassKernelResults."""
    import profhook

    profhook.install()
    from concourse.bass_utils import run_bass_kernel_spmd

    nc, in_maps = _CACHE["last"]
    res = run_bass_kernel_spmd(nc, in_maps, list(range(N_CORES)), trace=True,
                               trace_cores=list(range(N_CORES)))
    return res
